# revision 1
# baseline (speedup 1.0000x reference)
"""NT-Xent loss kernel for Trainium2 (8 NeuronCores, Bass/Tile).

Strategy (see sharding hint): rows of the 2Nx2N similarity matrix are
sharded across the 8 cores.  Host-side we only do data marshalling:
z = concat(z1, z2) and each core receives np.roll(z, -1024*c, axis=0)
so that the SPMD kernel always works on rows [0, 1024) of its rotated
view (row permutation leaves each row's logsumexp unchanged, maps the
diagonal to the diagonal, and maps the positive-pair column to the
static range [4096, 5120)).

On-device per core:
  1. DMA the full rotated z [8192, 256] fp32.
  2. Row norms on DVE: fused square+row-sum (tensor_tensor_reduce),
     ACT sqrt -> max(eps) -> DVE reciprocal -> r.
  3. Normalize fp32->bf16 (per-partition tensor_scalar), PE-transpose
     128x128 blocks into PSUM (bf16), DVE-copy into znT [2x128, 8192].
  4. For its 8 row-tiles x 4 col-chunks: 512-wide bf16 matmuls
     (K=256 accumulated in PSUM fp32), then one ACT Exp(scale=10)
     over [128, 2048] PSUM with accum_out -> fused row sums.
  5. lse = Ln(rowsum - exp(10*diag)), pos = 10 * <zn_i, zn_{i+4096}>.
  6. Output [128, 2] per-partition partial sums of (lse, pos).

Host combines: loss = (sum(lse) - sum(pos)) / 8192.
"""

import sys

if "/opt/trn_rl_repo" not in sys.path:
    sys.path.insert(0, "/opt/trn_rl_repo")

import numpy as np

import concourse.bacc as bacc
import concourse.mybir as mybir
import concourse.tile as tile
from concourse.masks import make_identity

P = 128
D = 256
M = 8192            # 2N rows
NCORES = 8
NT = M // P         # 64 row tiles of the full z
IT = (M // NCORES) // P   # 8 row tiles owned per core
TEMP_INV = 10.0     # 1 / temperature
EPS = 1e-8
F32 = mybir.dt.float32
BF16 = mybir.dt.bfloat16
FP8 = mybir.dt.float8e5
CHUNK = 2048        # columns of sim handled per PSUM tile / ACT pass
NCH = M // CHUNK    # 4 col chunks per row tile
NSUB = CHUNK // 512

_nc_cache = None


def _build():
    nc = bacc.Bacc(None, target_bir_lowering=False)
    z = nc.dram_tensor("z", [M, D], F32, kind="ExternalInput")
    out = nc.dram_tensor("out", [P, 2], F32, kind="ExternalOutput")

    AF = mybir.ActivationFunctionType
    ALU = mybir.AluOpType

    with (
        tile.TileContext(nc) as tc,
        tc.tile_pool(name="big", bufs=1) as big,
        tc.tile_pool(name="small", bufs=1) as small,
        tc.tile_pool(name="zpool", bufs=16) as zpool,
        tc.tile_pool(name="psp", bufs=2, space="PSUM") as psp,
    ):
        znn = big.tile([P, NT, D], BF16)     # normalized z (natural layout)
        znT = big.tile([P, 2, M], BF16)      # normalized z transposed
        # Dead output buffers (data never read; only accum_out matters).
        # ACT encodes a single sync-wait per instruction, so ACT ops write
        # never-reused subtiles; DVE TTR outs use stride-0 broadcast dests.
        exp_dead = big.tile([P, 16, CHUNK], FP8)
        sq_dead = big.tile([P, NT, D], BF16)
        dot_dead = big.tile([P, 2 * IT, D], F32)
        ss = small.tile([P, NT], F32)        # row norm^2
        rn = small.tile([P, NT], F32)        # 1 / max(norm, eps)
        ident = small.tile([P, P], BF16)
        make_identity(nc, ident)

        zv = z.rearrange("(t p) d -> p t d", p=P)

        # ---- load + norms + normalize, in groups of 8 row tiles ----
        GK = 8
        for g in range(NT // GK):
            ztiles = []
            for j in range(GK):
                t = g * GK + j
                zrt = zpool.tile([P, D], F32, tag="zrt", name=f"zrt_{t}")
                (nc.sync if j % 2 == 0 else nc.gpsimd).dma_start(out=zrt, in_=zv[:, t, :])
                ztiles.append(zrt)
            for j in range(GK):
                t = g * GK + j
                nc.scalar.activation(
                    out=sq_dead[:, t, :],
                    in_=ztiles[j],
                    func=AF.Square,
                    accum_out=ss[:, t : t + 1],
                )
            sl = slice(g * GK, (g + 1) * GK)
            nc.scalar.activation(rn[:, sl], ss[:, sl], AF.Sqrt)
            nc.vector.tensor_scalar_max(rn[:, sl], rn[:, sl], EPS)
            nc.vector.reciprocal(rn[:, sl], rn[:, sl])
            for j in range(GK):
                t = g * GK + j
                nc.vector.tensor_scalar_mul(
                    znn[:, t, :], ztiles[j], rn[:, t : t + 1]
                )
            # PE-transpose this group's 8 tiles (16 [128,128] blocks)
            pt = psp.tile([P, 2, GK, P], BF16, tag="ps", name=f"pt_{g}")
            for j in range(GK):
                t = g * GK + j
                for k in range(2):
                    nc.tensor.transpose(
                        pt[:, k, j, :], znn[:, t, k * P : (k + 1) * P], ident
                    )
            for k in range(2):
                nc.vector.tensor_copy(
                    out=znT[:, k, g * (GK * P) : (g + 1) * (GK * P)],
                    in_=pt[:, k].rearrange("p j c -> p (j c)"),
                )

        # ---- tail dot products (early, overlap with main loop) ----
        dd = small.tile([P, IT], F32)   # <zn_i, zn_i>
        pp = small.tile([P, IT], F32)   # 10 * <zn_i, zn_{i+4096}>
        for i in range(IT):
            nc.vector.tensor_mul(dot_dead[:, i, :], znn[:, i, :], znn[:, i, :])
            nc.vector.reduce_sum(
                dd[:, i : i + 1], dot_dead[:, i, :], axis=mybir.AxisListType.X
            )
            nc.vector.tensor_mul(
                dot_dead[:, IT + i, :], znn[:, i, :], znn[:, 4 * IT + i, :]
            )
            nc.vector.reduce_sum(
                pp[:, i : i + 1], dot_dead[:, IT + i, :],
                axis=mybir.AxisListType.X,
            )
        nc.vector.tensor_scalar_mul(pp, pp, TEMP_INV)

        # ---- main loop: sim row-block x col-chunk, fused exp row sums ----
        acc = small.tile([P, IT, NCH], F32)
        for i in range(IT):
            for c in range(NCH):
                ps = psp.tile([P, CHUNK], F32, tag="ps", name=f"ps_{i}_{c}")
                for k in range(2):
                    for n in range(NSUB):
                        nc.tensor.matmul(
                            ps[:, n * 512 : (n + 1) * 512],
                            lhsT=znT[:, k, i * P : (i + 1) * P],
                            rhs=znT[
                                :, k, c * CHUNK + n * 512 : c * CHUNK + (n + 1) * 512
                            ],
                            start=(k == 0),
                            stop=(k == 1),
                        )
                nc.scalar.activation(
                    out=exp_dead[:, (i * NCH + c) % 16, :],
                    in_=ps[:],
                    func=AF.Exp,
                    scale=TEMP_INV,
                    accum_out=acc[:, i, c : c + 1],
                )

        # ---- tail: lse and output ----
        rowsum = small.tile([P, IT], F32)
        nc.vector.reduce_sum(rowsum, acc, axis=mybir.AxisListType.X)
        ed = small.tile([P, IT], F32)
        nc.scalar.activation(ed, dd, AF.Exp, scale=TEMP_INV)
        nc.vector.tensor_sub(rowsum, rowsum, ed)
        lse = small.tile([P, IT], F32)
        nc.scalar.activation(lse, rowsum, AF.Ln)

        outs = small.tile([P, 2], F32)
        nc.vector.reduce_sum(outs[:, 0:1], lse, axis=mybir.AxisListType.X)
        nc.vector.reduce_sum(outs[:, 1:2], pp, axis=mybir.AxisListType.X)
        nc.sync.dma_start(out=out[:], in_=outs)

    nc.finalize()
    return nc


def _get_nc():
    global _nc_cache
    if _nc_cache is None:
        _nc_cache = _build()
    return _nc_cache


def _run_cores(z: np.ndarray, trace: bool = False):
    """Run the SPMD kernel on 8 cores. Returns per-core results + perf."""
    from concourse.bass_utils import run_bass_kernel_spmd

    nc = _get_nc()
    rows_per_core = M // NCORES
    in_maps = [
        {"z": np.ascontiguousarray(np.roll(z, -rows_per_core * c, axis=0))}
        for c in range(NCORES)
    ]
    res = run_bass_kernel_spmd(
        nc, in_maps, core_ids=list(range(NCORES)), trace=trace
    )
    return res


def kernel(z1: np.ndarray, z2: np.ndarray) -> np.ndarray:
    z = np.concatenate(
        [np.asarray(z1, np.float32), np.asarray(z2, np.float32)], axis=0
    )
    res = _run_cores(z)
    parts = np.stack([r["out"] for r in res.results]).astype(np.float64)
    lse_sum = parts[:, :, 0].sum()
    pos_sum = parts[:, :, 1].sum()
    return np.float32((lse_sum - pos_sum) / M)

